# revision 1
# baseline (speedup 1.0000x reference)
"""Trainium2 Bass kernel for DietConv2dV2: 3x3 conv (stride 1, pad 1) + bias.

x: (16, 8, 1024, 1024) fp32, weight: (8, 8, 3, 3), bias: (8,) -> out like x.

Strategy
--------
Data-parallel: 16 images / 8 cores = 2 images per core, no collectives.

Per core the conv runs as a banded matmul on the PE array:
  - K (contraction, partitions) = 16 input rows x 8 in-channels = 128,
    partition p = r*8 + ci.  (A channel-major p = ci*16 + r, which
    makes the HBM reads contiguous 64KB runs, was tried and LOST ~70us:
    the SWDGE cast path emits a 4-byte companion descriptor per 4KB
    data descriptor when consecutive descriptors have contiguous
    sources -- 21k extra 20ns descriptor slots flood the SDMA engines.)
  - M (stationary free dim)     = 8 out-channels x 14 out rows = 112,
    column  m = co*14 + ho (channel-major: consecutive output
    descriptors then write contiguous HBM, and the HWDGE path has no
    companion-descriptor pathology).
  - N (moving free dim)         = 512-wide w chunk (PSUM bank).
The stationary "band" matrix S_kw[(r,ci),(co,ho)] = weight[co,ci,r-ho,kw]
covers all 3 kh taps at once; the 3 kw taps are 3 PSUM-accumulated
matmuls reading the same SBUF rows at w offsets kw (rows stored with a
1-col zero pad on each side).  Band matrices are precomputed on the
host from `weight` (host preprocessing of a 2.3KB tensor) and loaded
once.

Inputs stream HBM->SBUF through the SWDGE cast path (fp32 -> bf16):
HBM reads stay fp32 (unavoidable), SBUF gets bf16.  bf16 matmuls
stream 1 column/cycle with half the SBUF port pressure of fp32r --
measured back-to-back cadence 216ns vs 283ns, and fp32r MMs stalled to
~473ns under DMA contention.  Bias is fused into the PSUM->SBUF
eviction, split per w-half across DVE (tensor_scalar add) and ACT
(Identity activation) so neither engine is the per-block critical
path; both convert to bf16.  Writing the output as bf16 halves store
traffic; the host upcasts the gathered result to fp32.  Total
quantization error ~2.9e-3 L2, well inside the 2e-2 gate.

Measured 386us/core (baseline fp32r+fp32-out: 432us).  Per-core HBM
traffic 111.6MB (77.6 in + 34 out) -> ~310us at the 358 GB/s/core
cap; the residual ~75us is DMA orchestration (packet round-robin,
sem descriptors, read/write turnaround) that resisted: single-queue
(-65us), HWDGE input (-118us), per-chunk PSUM (race), and c-major
input (-70us) all made it worse; see inline notes.

The last row-block is shifted up to start at h=1010 so every block
writes a full 14 rows (rows 1010..1021 are written twice with identical
bytes); this keeps one uniform [112, W] eviction + store path.
"""

import numpy as np

import bass_rust
import concourse.bass as bass
import concourse.mybir as mybir
from concourse.tile import TileContext
from concourse.bass_utils import run_bass_kernel_spmd

F32 = mybir.dt.float32
F32R = mybir.dt.float32r
BF16 = mybir.dt.bfloat16

N_CORES = 8
IMG_PER_CORE = 2
C = 8          # channels (in == out)
H = 1024
W = 1024
KS = 3         # kernel size
HB = 14        # output rows per block (16 input rows -> 14 output rows)
KROWS = HB + KS - 1  # 16 input rows per block
M = C * HB     # 112 stationary columns
WCHUNK = 512   # PSUM bank = 512 fp32
PADL = 16      # data offset in xt: 16 bf16 cols = 32B-aligned DMA landing


def _split_excess_waits(nc):
    """This walrus build accepts 1 sync-wait per instruction (2 for
    EventSemaphore); Tile's final drain and ldweights can end up with
    more.  Move overflow waits onto EventSemaphore carriers inserted
    before the offender on the same engine."""
    for fn in nc.m.functions:
        for blk in fn.blocks:
            out = []
            changed = False
            for inst in blk.instructions:
                si = inst.sync_info
                cap = 2 if inst.opcode == "EventSemaphore" else 1
                waits = list(si.on_wait) if si is not None else []
                if len(waits) > cap:
                    changed = True
                    overflow, keep = waits[:-cap], waits[-cap:]
                    for j in range(0, len(overflow), 2):
                        es = mybir.InstEventSemaphore(
                            name=nc.get_next_instruction_name(), ins=[], outs=[]
                        )
                        es.engine = inst.engine
                        es.sync_info = bass_rust.SyncInfo(
                            on_wait=overflow[j : j + 2], on_update=[]
                        )
                        nc.register_instruction(es, overwrite=True)
                        out.append(es)
                    inst.sync_info = bass_rust.SyncInfo(
                        on_wait=keep, on_update=list(si.on_update)
                    )
                out.append(inst)
            if changed:
                blk.instructions = out


def _block_starts(h):
    """Full-HB block starts covering [0, h): 0,14,...; the last block is
    shifted up so it still spans HB full rows."""
    starts = list(range(0, h - HB + 1, HB))
    if starts[-1] + HB < h:
        starts.append(h - HB)
    return starts


def _build(nimg, h, w, reps=1, salt=0):
    nchunks = w // WCHUNK
    starts = _block_starts(h)

    nc = bass.Bass(name=f"dietconv_s{salt}")
    x = nc.dram_tensor("x", [nimg, C, h, w], F32, kind="ExternalInput")
    wb = nc.dram_tensor("wband", [KS, 128, M], BF16, kind="ExternalInput")
    bv = nc.dram_tensor("biasv", [M, 1], F32, kind="ExternalInput")
    out = nc.dram_tensor("out", [nimg, C, h, w], BF16, kind="ExternalOutput")

    # row-major (h, c) view so SBUF partition p = r*8 + ci
    xr = x.rearrange("n c h w -> n h c w")

    with TileContext(nc) as tc:
        with (
            tc.tile_pool(name="wpool", bufs=1) as wpool,
            tc.tile_pool(name="xpool", bufs=8) as xpool,
            tc.tile_pool(name="opool", bufs=10) as opool,
            tc.tile_pool(name="pspool", bufs=4, space="PSUM") as pspool,
        ):
            wts = []
            for kw in range(KS):
                wt = wpool.tile([128, M], BF16, name=f"wt{kw}")
                nc.sync.dma_start(out=wt[:], in_=wb[kw])
                wts.append(wt)
            bt = wpool.tile([M, 1], F32, name="bt")
            nc.sync.dma_start(out=bt[:], in_=bv[:])

            def body():
                for n in range(nimg):
                    for b, h0 in enumerate(starts):
                        hlo = h0 - 1  # input rows [hlo, hlo + KROWS)
                        vlo = max(hlo, 0)
                        vhi = min(hlo + KROWS, h)
                        plo = (vlo - hlo) * C
                        phi = (vhi - hlo) * C
                        # col c holds input w = c-1; cols 0 and w+1 are
                        # zero padding (fp32r matmuls need even N and
                        # 8B-aligned PSUM offsets, so clipping edge taps
                        # is not an option -- pad instead).
                        # (An fp32 HWDGE load + on-chip DVE/ACT cast to
                        # bf16 was tried to bypass SWDGE: 507us, -118us.
                        # HWDGE serializes per-DMA completion on its FIFO
                        # ring; the SWDGE cast path pipelines bulk input
                        # far better.)
                        # data lands at col PADL (32B-aligned for clean
                        # 2KB S2M writes); cols PADL-1 and PADL+w are the
                        # zero pads the kw taps read
                        xt = xpool.tile([128, w + 2 * PADL], BF16, name="xt")
                        nc.vector.memset(xt[:, PADL - 1 : PADL], 0.0)
                        nc.vector.memset(xt[:, PADL + w : PADL + w + 1], 0.0)
                        # zero out-of-image rows.  DVE partition start must
                        # be 32-aligned, so memset a wider aligned range;
                        # the DMA below rewrites the valid rows (Tile
                        # serializes the WAW overlap).
                        if plo > 0:
                            nc.vector.memset(xt[0:plo, :], 0.0)
                        if phi < 128:
                            alo = (phi // 32) * 32
                            nc.vector.memset(xt[alo:128, :], 0.0)
                        # NOTE: replacing the 2-row HBM re-read (2/16 of
                        # input traffic, ~26us) with an SBUF->SBUF halo
                        # copy from the previous block was tried twice
                        # (dedicated HWDGE ring included) and lost
                        # ~140us: Tile orders the copy against the same
                        # tile's main load, serializing the load pipeline
                        # on DMA completion latency.  The re-read overlaps
                        # freely and wins.
                        nc.gpsimd.dma_start(
                            out=xt[plo:phi, PADL : PADL + w],
                            in_=xr[n, vlo:vhi, :, :],
                        )
                        # one PSUM tile spanning both w-chunks (2 banks);
                        # each matmul stays within one bank.  (Per-chunk
                        # PSUM tiles with per-chunk eviction were tried:
                        # no speedup and an intermittent-NaN race.)
                        ps = pspool.tile([M, w], F32, name="ps", tag="ps")
                        for j in range(nchunks):
                            base = j * WCHUNK
                            # kw tap reads tile col wo + kw (= input w + 1)
                            for kw in range(KS):
                                c0 = base + PADL - 1 + kw
                                nc.tensor.matmul(
                                    ps[:, base : base + WCHUNK],
                                    wts[kw][:],
                                    xt[:, c0 : c0 + WCHUNK],
                                    start=(kw == 0),
                                    stop=(kw == KS - 1),
                                )
                        ot = opool.tile([M, w], BF16, name="ot", tag="ot")
                        # split PSUM->SBUF eviction across DVE and ACT so
                        # neither engine becomes the per-block critical path
                        half = w // 2
                        nc.vector.tensor_scalar_add(
                            ot[:, 0:half], ps[:, 0:half], bt[:]
                        )
                        nc.scalar.activation(
                            ot[:, half:w],
                            ps[:, half:w],
                            mybir.ActivationFunctionType.Identity,
                            bias=bt[:],
                        )
                        # alternate output DMAs across both HWDGE rings
                        # (sync + scalar): ~70us faster than one ring;
                        # routing outputs onto the SWDGE queue instead
                        # (single-queue everything) lost ~65us
                        dma_eng = nc.sync if b % 2 == 0 else nc.scalar
                        dma_eng.dma_start(
                            out=out[n, :, h0 : h0 + HB, :],
                            in_=ot[:],
                        )

            # static unroll: tc.For_i loop control hits a walrus codegen
            # gap in this build ("ISA wrong length" on CompareAndBranch)
            for _ in range(reps):
                body()

    _split_excess_waits(nc)
    return nc


def _band_inputs(weight, bias):
    weight = np.asarray(weight, dtype=np.float32)
    bias = np.asarray(bias, dtype=np.float32)
    S = np.zeros((KS, 128, M), dtype=np.float32)  # cast to bf16 at the end
    for kw in range(KS):
        for kh in range(KS):
            for ho in range(HB):
                r = ho + kh
                for ci in range(C):
                    for co in range(C):
                        S[kw, r * C + ci, co * HB + ho] = weight[co, ci, kh, kw]
    biasv = np.repeat(bias, HB).astype(np.float32)[:, None]  # m = co*14 + ho
    import concourse.mybir as _mybir

    return S.astype(_mybir.dt.np(BF16)), biasv


def _run(x, weight, bias, nimg_per_core, h, w, n_cores, reps=1):
    S, biasv = _band_inputs(weight, bias)
    x = np.ascontiguousarray(x, dtype=np.float32)
    in_maps = [
        {
            "x": x[i * nimg_per_core : (i + 1) * nimg_per_core],
            "wband": S,
            "biasv": biasv,
        }
        for i in range(n_cores)
    ]
    # The walrus backend compile is rarely flaky (parallel codegen race).
    # jax caches the failed compilation by HLO, so retries must change the
    # BIR bytes (salt) and drop the jit cache.
    last_exc = None
    for attempt in range(4):
        try:
            nc = _build(nimg_per_core, h, w, reps, salt=attempt)
            res = run_bass_kernel_spmd(nc, in_maps, core_ids=list(range(n_cores)))
            break
        except Exception as e:  # noqa: BLE001
            last_exc = e
            try:
                import jax

                jax.clear_caches()
            except Exception:  # noqa: BLE001
                pass
    else:
        raise last_exc
    return np.concatenate(
        [np.asarray(r["out"]).astype(np.float32) for r in res.results], axis=0
    )


def kernel(x, weight, bias):
    return _run(x, weight, bias, IMG_PER_CORE, H, W, N_CORES, reps=1)

